# revision 11
# baseline (speedup 1.0000x reference)
"""Trainium2 Bass kernel for nn_CrossAttention (B=4, Sx=Sy=2048, D=1024, f32).

Sharding: data-parallel over (batch b, query-half h) -> 8 cores; each core
computes full cross-attention for 1024 query rows of one batch against all
2048 keys of that batch (K/V projections duplicated across the 2 cores
sharing a batch; no collectives -- the collective cost model makes any
K/V exchange slower than recomputing).

Every GEMM runs as fp8e4 DoubleRow matmuls (K=256 per instruction, 0.5
cycles/row -> 4x f32r throughput) with a 3-term hi/lo expansion for
bf16-class accuracy:  A@B ~= Ah@Bh + Ah@Bl + Al@Bh  where Ah = fp8(A),
Al = fp8(A - Ah).  x/y/W are split on the host (weights pre-scaled by 8 to
clear the fp8 subnormal range; descaled in the PSUM post-op).  Q/K/V/P are
split on device: ACT writes T = ps*0.125 + bias (fp16), DVE derives
hi = fp8(T), lo = fp8(T - hi).

Per-core pipeline:
  P1: Qps[e,s]  = Wq'^T x xT          -> T -> QTh/QTl   (fp8 hi/lo)
  P2: Kps[e,t]  = Wk'^T x yT          -> T -> KTh/KTl
  P3: Vps[t,e]  = yT^T x Wv'          -> T -> Vh/Vl     (fused chunk loop
      with P2 so each y chunk is loaded once)
  P4 (per 256-wide s-block): Sps[t,s] = KT^T @ QT (3-term)
      E = exp(Sps/32 - 1)  (fp16)     -> PTh/PTl
  P5: num[s,e] = PT^T @ V (3-term), den[s] = PT^T @ ones,
      out = num * (1/den)
Softmax normalization cancels the exp(-1) shift and the 1/32 score scale
is applied inside the exp activation, so Q/K stay at unit scale in fp8.
"""

import numpy as np
import ml_dtypes

import concourse.bacc as bacc
import concourse.bass as bass
import concourse.tile as tile
import concourse.mybir as mybir
from concourse.bass_utils import run_bass_kernel_spmd

F32 = mybir.dt.float32
F16 = mybir.dt.float16
F8 = mybir.dt.float8e4
DR = mybir.MatmulPerfMode.DoubleRow
E4 = ml_dtypes.float8_e4m3
AF = mybir.ActivationFunctionType

B, SX, SY, D = 4, 2048, 2048, 1024
NCORES = 8
SXH = SX // 2          # query rows per core
DB = D // 128          # contraction blocks of 128
EB = D // 128          # output-feature blocks
TBLK = SY // 128       # key blocks of 128
SSB = 256              # s-superblock width (one PSUM tile of scores per tb)
CH = 256               # activation staging chunk (tokens)
WSCALE = 8.0           # host premultiplier on weights (fp8 subnormal guard)

_CACHE = {}


def _build():
    nc = bacc.Bacc("TRN2", target_bir_lowering=False, debug=False,
                   num_devices=NCORES, dynamic_dma_scratch_size=2048)

    xh_d = nc.dram_tensor("xh", [DB, 128, SXH], F8, kind="ExternalInput").ap()
    xl_d = nc.dram_tensor("xl", [DB, 128, SXH], F8, kind="ExternalInput").ap()
    yh_d = nc.dram_tensor("yh", [DB, 128, SY], F8, kind="ExternalInput").ap()
    yl_d = nc.dram_tensor("yl", [DB, 128, SY], F8, kind="ExternalInput").ap()
    w_d = {}
    for w in ("wq", "wk", "wv"):
        for p in ("h", "l"):
            w_d[w + p] = nc.dram_tensor(
                w + p, [DB, 128, D], F8, kind="ExternalInput").ap()
    bq_d = nc.dram_tensor("bq2", [EB, 128], F32, kind="ExternalInput").ap()
    bk_d = nc.dram_tensor("bk2", [EB, 128], F32, kind="ExternalInput").ap()
    out_d = nc.dram_tensor("out", [SXH, D], F32, kind="ExternalOutput").ap()

    with tile.TileContext(nc) as tc:
        with (
            tc.tile_pool(name="misc", bufs=1) as misc,
            tc.tile_pool(name="tst", bufs=4) as tst,       # fp16 T staging
            tc.tile_pool(name="est", bufs=4) as est,       # fp16 E staging
            tc.tile_pool(name="ostage", bufs=3) as ostage,
            tc.tile_pool(name="rstage", bufs=2) as rstage,
            tc.tile_pool(name="ps_big", bufs=6, space="PSUM") as ps_big,
            tc.tile_pool(name="ps_sum", bufs=2, space="PSUM") as ps_sum,
            tc.tile_pool(name="persist", bufs=1) as persist,
            tc.tile_pool(name="ptp", bufs=4) as ptp,
            tc.tile_pool(name="wst", bufs=4) as wst,
            tc.tile_pool(name="ast", bufs=8) as ast,
        ):
            bq_t = misc.tile([128, EB], F32)
            bk_t = misc.tile([128, EB], F32)
            ones_f = misc.tile([128, 2, 2], F32)
            ones_t = misc.tile([128, 2, 2], F8)
            nc.vector.memset(ones_f, 1.0)
            nc.vector.tensor_copy(out=ones_t, in_=ones_f)
            negone = misc.tile([128, 1], F32)
            nc.vector.memset(negone, -2.0)

            QTh = persist.tile([128, EB, SXH], F8)     # 8 KB/part
            QTl = persist.tile([128, EB, SXH], F8)
            KTh = persist.tile([128, EB, SY], F8)      # 16 KB/part
            KTl = persist.tile([128, EB, SY], F8)
            Vh = persist.tile([128, TBLK, D], F8)      # 16 KB/part
            Vl = persist.tile([128, TBLK, D], F8)

            def mm3(ps, lh, ll, rh, rl, nk, accum=False):
                """3-term hi/lo DR accumulation into ps over nk 256-blocks.

                lh/ll/rh/rl: callables j -> AP ([128, 2, M] / [128, 2, N])."""
                terms = [(lh, rh), (lh, rl), (ll, rh)]
                n = 3 * nk
                i = 0
                for (lf, rf) in terms:
                    for j in range(nk):
                        nc.tensor.matmul(
                            ps, lhsT=lf(j), rhs=rf(j),
                            start=(i == 0 and not accum),
                            stop=(i == n - 1),
                            perf_mode=DR)
                        i += 1

            def load_w(name, pieces=None):
                wt = wst.tile([128, DB, D], F8, tag="w")
                for lo, hi in (pieces or [(0, D)]):
                    nc.sync.dma_start(
                        out=wt[:, :, lo:hi],
                        in_=w_d[name][:, :, lo:hi]
                        .rearrange("db p e -> p db e"))
                return wt

            def load_act(src_h, src_l, t0, width):
                ah = ast.tile([128, DB, width], F8, tag="a")
                al = ast.tile([128, DB, width], F8, tag="a")
                nc.sync.dma_start(
                    out=ah, in_=src_h[:, :, t0:t0 + width]
                    .rearrange("db p s -> p db s"))
                nc.sync.dma_start(
                    out=al, in_=src_l[:, :, t0:t0 + width]
                    .rearrange("db p s -> p db s"))
                return ah, al

            # ---- P1: Q projection ----
            # DMA issue order tuned so the first matmul's operands land first
            # (wq piece 0 + x chunk 0), then P2's y chunk 0 ahead of the
            # wk/wv prefetches.
            pieces = [(0, 128), (128, 512), (512, D)]
            wq_h = load_w("wqh", pieces=[(0, 128)])
            wq_l = load_w("wql", pieces=[(0, 128)])
            xc0 = load_act(xh_d, xl_d, 0, CH)
            for lo, hi in pieces[1:]:
                nc.sync.dma_start(out=wq_h[:, :, lo:hi],
                                  in_=w_d["wqh"][:, :, lo:hi]
                                  .rearrange("db p e -> p db e"))
                nc.sync.dma_start(out=wq_l[:, :, lo:hi],
                                  in_=w_d["wql"][:, :, lo:hi]
                                  .rearrange("db p e -> p db e"))
            nc.sync.dma_start(out=bq_t, in_=bq_d.rearrange("eb p -> p eb"))
            nc.sync.dma_start(out=bk_t, in_=bk_d.rearrange("eb p -> p eb"))
            # P2's first y chunk ahead of the Wk/Wv prefetches
            yc0 = load_act(yh_d, yl_d, 0, CH)
            wk_h, wk_l = load_w("wkh"), load_w("wkl")
            wv_h, wv_l = load_w("wvh"), load_w("wvl")

            def proj_post(ps, Th_dst, Tl_dst, bias, width=CH):
                """T = ps*0.125 + bias (ACT, fp16); hi = fp8(T), lo = T - hi."""
                t = tst.tile([128, width], F16, tag="t")
                if bias is None:
                    nc.scalar.activation(out=t, in_=ps, func=AF.Copy,
                                         scale=1.0 / WSCALE)
                else:
                    nc.scalar.activation(out=t, in_=ps, func=AF.Identity,
                                         scale=1.0 / WSCALE, bias=bias)
                nc.vector.tensor_copy(out=Th_dst, in_=t)
                nc.gpsimd.tensor_tensor(
                    out=Tl_dst, in0=t, in1=Th_dst,
                    op=mybir.AluOpType.subtract)

            for ci in range(SXH // CH):
                s0 = ci * CH
                xc = xc0 if ci == 0 else load_act(xh_d, xl_d, s0, CH)
                xch, xcl = xc
                for eb in range(EB):
                    ps = ps_big.tile([128, CH], F32, tag="ps")
                    c0, c1 = eb * 128, (eb + 1) * 128
                    mm3(ps,
                        lambda j: wq_h[:, 2 * j:2 * j + 2, c0:c1],
                        lambda j: wq_l[:, 2 * j:2 * j + 2, c0:c1],
                        lambda j: xch[:, 2 * j:2 * j + 2, :],
                        lambda j: xcl[:, 2 * j:2 * j + 2, :],
                        DB // 2)
                    proj_post(ps, QTh[:, eb, s0:s0 + CH], QTl[:, eb, s0:s0 + CH],
                              bq_t[:, eb:eb + 1])

            # ---- P2+P3 fused: K and V projections share y chunks ----
            for ci in range(SY // CH):
                t0 = ci * CH
                yc = yc0 if ci == 0 else load_act(yh_d, yl_d, t0, CH)
                ych, ycl = yc
                for eb in range(EB):
                    ps = ps_big.tile([128, CH], F32, tag="ps")
                    c0, c1 = eb * 128, (eb + 1) * 128
                    mm3(ps,
                        lambda j: wk_h[:, 2 * j:2 * j + 2, c0:c1],
                        lambda j: wk_l[:, 2 * j:2 * j + 2, c0:c1],
                        lambda j: ych[:, 2 * j:2 * j + 2, :],
                        lambda j: ycl[:, 2 * j:2 * j + 2, :],
                        DB // 2)
                    proj_post(ps, KTh[:, eb, t0:t0 + CH], KTl[:, eb, t0:t0 + CH],
                              bk_t[:, eb:eb + 1])
                for tbi in range(CH // 128):
                    tb = ci * (CH // 128) + tbi
                    tl0, tl1 = tbi * 128, (tbi + 1) * 128
                    for eh in range(D // 256):
                        e0 = eh * 256
                        ps = ps_big.tile([128, 256], F32, tag="ps")
                        mm3(ps,
                            lambda j: ych[:, 2 * j:2 * j + 2, tl0:tl1],
                            lambda j: ycl[:, 2 * j:2 * j + 2, tl0:tl1],
                            lambda j: wv_h[:, 2 * j:2 * j + 2, e0:e0 + 256],
                            lambda j: wv_l[:, 2 * j:2 * j + 2, e0:e0 + 256],
                            DB // 2)
                        proj_post(ps, Vh[:, tb, e0:e0 + 256],
                                  Vl[:, tb, e0:e0 + 256], None)

            # ---- P4+P5: attention per 256-wide s-superblock ----
            for ssb in range(SXH // SSB):
                s0 = ssb * SSB
                PTh = ptp.tile([128, TBLK, SSB], F8, tag="pth")
                PTl = ptp.tile([128, TBLK, SSB], F8, tag="ptl")
                for tb in range(TBLK):
                    ps = ps_big.tile([128, SSB], F32, tag="ps")
                    t0, t1 = tb * 128, (tb + 1) * 128
                    mm3(ps,
                        lambda j: KTh[:, 2 * j:2 * j + 2, t0:t1],
                        lambda j: KTl[:, 2 * j:2 * j + 2, t0:t1],
                        lambda j: QTh[:, 2 * j:2 * j + 2, s0:s0 + SSB],
                        lambda j: QTl[:, 2 * j:2 * j + 2, s0:s0 + SSB],
                        EB // 2)
                    e = est.tile([128, SSB], F16, tag="e")
                    nc.scalar.activation(out=e, in_=ps, func=AF.Exp,
                                         scale=1.0 / 32.0, bias=negone[:, 0:1])
                    nc.vector.tensor_copy(out=PTh[:, tb, :], in_=e)
                    nc.vector.tensor_tensor(
                        out=PTl[:, tb, :], in0=e, in1=PTh[:, tb, :],
                        op=mybir.AluOpType.subtract)
                for sbi in range(SSB // 128):
                    sl = sbi * 128
                    pss = ps_sum.tile([128, 2], F32, tag="pss")
                    i = 0
                    for pt in (PTh, PTl):
                        for j in range(TBLK // 2):
                            nc.tensor.matmul(
                                pss, lhsT=pt[:, 2 * j:2 * j + 2, sl:sl + 128],
                                rhs=ones_t,
                                start=(i == 0), stop=(i == TBLK - 1),
                                perf_mode=DR)
                            i += 1
                    rec = rstage.tile([128, 1], F32, tag="rec")
                    nc.vector.reciprocal(rec, pss[:, 0:1])
                    for eh in range(D // 256):
                        e0 = eh * 256
                        ps = ps_big.tile([128, 256], F32, tag="ps")
                        mm3(ps,
                            lambda j: PTh[:, 2 * j:2 * j + 2, sl:sl + 128],
                            lambda j: PTl[:, 2 * j:2 * j + 2, sl:sl + 128],
                            lambda j: Vh[:, 2 * j:2 * j + 2, e0:e0 + 256],
                            lambda j: Vl[:, 2 * j:2 * j + 2, e0:e0 + 256],
                            TBLK // 2)
                        o = ostage.tile([128, 256], F32, tag="o")
                        nc.vector.tensor_scalar_mul(
                            out=o, in0=ps, scalar1=rec[:, 0:1])
                        nc.sync.dma_start(
                            out=out_d[s0 + sl:s0 + sl + 128, e0:e0 + 256],
                            in_=o)

    nc.compile()
    return nc


def _get_nc():
    if "nc" not in _CACHE:
        _CACHE["nc"] = _build()
    return _CACHE["nc"]


def _hl(a):
    h = a.astype(E4)
    l = (a - h.astype(np.float32)).astype(E4)
    return h, l


def make_in_maps(x, y, Wq, bq, Wk, bk, Wv, bv):
    x = np.asarray(x, dtype=np.float32)
    y = np.asarray(y, dtype=np.float32)
    wsplit = {}
    for name, W in (("wq", Wq), ("wk", Wk), ("wv", Wv)):
        h, l = _hl(np.asarray(W, dtype=np.float32) * WSCALE)
        wsplit[name + "h"] = np.ascontiguousarray(h.reshape(DB, 128, D))
        wsplit[name + "l"] = np.ascontiguousarray(l.reshape(DB, 128, D))
    bq2 = np.ascontiguousarray(np.asarray(bq, dtype=np.float32).reshape(EB, 128))
    bk2 = np.ascontiguousarray(np.asarray(bk, dtype=np.float32).reshape(EB, 128))

    yhs, yls = {}, {}
    for b in range(B):
        h, l = _hl(y[b].T)
        yhs[b] = np.ascontiguousarray(h).reshape(DB, 128, SY)
        yls[b] = np.ascontiguousarray(l).reshape(DB, 128, SY)

    in_maps = []
    for c in range(NCORES):
        b, hh = divmod(c, 2)
        xt = x[b, hh * SXH:(hh + 1) * SXH, :].T
        xh, xl = _hl(np.ascontiguousarray(xt))
        in_maps.append({
            "xh": xh.reshape(DB, 128, SXH), "xl": xl.reshape(DB, 128, SXH),
            "yh": yhs[b], "yl": yls[b],
            "bq2": bq2, "bk2": bk2, **wsplit,
        })
    return in_maps


def assemble(results, bv):
    bv = np.asarray(bv, dtype=np.float32)
    out = np.empty((B, SX, D), dtype=np.float32)
    for c in range(NCORES):
        b, h = divmod(c, 2)
        out[b, h * SXH:(h + 1) * SXH, :] = results[c]["out"]
    out += bv[None, None, :]
    return out


def kernel(x, y, Wq, bq, Wk, bk, Wv, bv):
    nc = _get_nc()
    in_maps = make_in_maps(x, y, Wq, bq, Wk, bk, Wv, bv)
    res = run_bass_kernel_spmd(nc, in_maps, list(range(NCORES)))
    return assemble(res.results, bv)
